# revision 2
# baseline (speedup 1.0000x reference)
"""Causal single-head attention (B=4, T=4096, C=1024, H=64) on 8 TRN2 cores.

Sharding: core = 2*b + h  (b = batch, h = kv-parity).  Each core computes,
for ALL queries of its batch, the partial softmax numerator and denominator
over the kv chunks (128 rows each) whose chunk index has parity h.  This
makes the per-core program identical across cores (SPMD requirement) and
perfectly load-balanced; the host combines partials:
    out = (num0 + num1) / (den0 + den1).

Device layout (per core):
  xq  [C, T]    = x[b].T (bf16)            -> Q projection, rhs of matmuls
  xkv [C, T/2]  = x[b].T parity-h columns  -> K,V projection
  Q^T [64, 512] per q-superblock; KV^T [128, 512] per kv-superblock
  S^T [128kv, 512q] = kT.T @ qT            (PE, contraction H=64)
  P^T = exp(S^T/32) (ACT) * causal mask (DVE, diagonal chunks only)
  outT [65, 512q] += Vaug.T @ P^T          (Vaug = [V | ones], PE)
  out rows via PE transpose of outT 128-col blocks.
"""

import numpy as np
import ml_dtypes

import concourse.bass as bass
import concourse.bacc as bacc
import concourse.tile as tile
from concourse import mybir
from concourse.bass_utils import run_bass_kernel_spmd

F32 = mybir.dt.float32
BF16 = mybir.dt.bfloat16

B = 4
C = 1024
H = 64
SUP = 512          # query superblock width
KC = 128           # kv chunk
CC = C // 128      # contraction chunks


def build_nc(T=4096, pt_bufs=4, xq_bufs=3, xkv_bufs=2, s_bufs=2):
    n_sup = T // SUP
    kv_cols = T // 2
    n_kv_sup = kv_cols // SUP
    scale = float(C) ** -0.5

    nc = bacc.Bacc(None, target_bir_lowering=False)
    xq_d = nc.dram_tensor("xq", [C, T], BF16, kind="ExternalInput")
    xkv_d = nc.dram_tensor("xkv", [C, kv_cols], BF16, kind="ExternalInput")
    wq_d = nc.dram_tensor("wq", [C, H], BF16, kind="ExternalInput")
    wkv_d = nc.dram_tensor("wkv", [C, 2 * H], BF16, kind="ExternalInput")
    bq_d = nc.dram_tensor("bq", [H, 1], F32, kind="ExternalInput")
    bkv_d = nc.dram_tensor("bkv", [2 * H, 1], F32, kind="ExternalInput")
    mask_d = nc.dram_tensor("mask", [128, 2, SUP], BF16, kind="ExternalInput")
    id_d = nc.dram_tensor("ident", [128, 128], F32, kind="ExternalInput")
    out_d = nc.dram_tensor("out", [T, H + 1], F32, kind="ExternalOutput")

    with tile.TileContext(nc) as tc:
        with (
            tc.tile_pool(name="consts", bufs=1) as consts,
            tc.tile_pool(name="xq", bufs=xq_bufs) as xqp,
            tc.tile_pool(name="xkv", bufs=xkv_bufs) as xkvp,
            tc.tile_pool(name="pers", bufs=1) as pers,
            tc.tile_pool(name="pt", bufs=pt_bufs) as ptp,
            tc.tile_pool(name="ot", bufs=2) as otsb,
            tc.tile_pool(name="stg", bufs=2) as stgp,
            tc.tile_pool(name="proj", bufs=2, space="PSUM") as projp,
            tc.tile_pool(name="spsum", bufs=s_bufs, space="PSUM") as sp,
            tc.tile_pool(name="otp", bufs=2, space="PSUM") as otp,
            tc.tile_pool(name="tpp", bufs=2, space="PSUM") as tpp,
        ):
            wq_sb = consts.tile([128, CC, H], BF16)
            nc.sync.dma_start(wq_sb[:], wq_d.rearrange("(cc p) m -> p cc m", p=128))
            wkv_sb = consts.tile([128, CC, 2 * H], BF16)
            nc.sync.dma_start(wkv_sb[:], wkv_d.rearrange("(cc p) m -> p cc m", p=128))
            bq_sb = consts.tile([H, 1], F32)
            nc.sync.dma_start(bq_sb[:], bq_d[:])
            bkv_sb = consts.tile([2 * H, 1], F32)
            nc.sync.dma_start(bkv_sb[:], bkv_d[:])
            mask_sb = consts.tile([128, 2, SUP], BF16)
            nc.sync.dma_start(mask_sb[:], mask_d[:])
            id_sb = consts.tile([128, 128], F32)
            nc.sync.dma_start(id_sb[:], id_d[:])

            xq_r = xq_d.rearrange("(cc p) t -> p cc t", p=128)
            xkv_r = xkv_d.rearrange("(cc p) t -> p cc t", p=128)

            # ---- K,V projection (packed: rows 0:64 = kT, 64:128 = vT) ----
            kvT = []
            vaug = []
            for s in range(n_kv_sup):
                xkv_sb = xkvp.tile([128, CC, SUP], BF16)
                nc.sync.dma_start(xkv_sb[:], xkv_r[:, :, s * SUP:(s + 1) * SUP])
                ps = projp.tile([128, SUP], F32, tag="proj")
                for c in range(CC):
                    nc.tensor.matmul(ps[:], wkv_sb[:, c, :], xkv_sb[:, c, :],
                                     start=(c == 0), stop=(c == CC - 1))
                kv_sb = pers.tile([128, SUP], BF16, tag=f"kvT{s}")
                nc.scalar.activation(kv_sb[:], ps[:],
                                     mybir.ActivationFunctionType.Identity,
                                     bias=bkv_sb[:], scale=1.0)
                kvT.append(kv_sb)
                for j in range(SUP // KC):
                    va = pers.tile([128, H + 1], BF16, tag=f"vaug{s * 4 + j}")
                    nc.vector.memset(va[:, H:H + 1], 1.0)
                    nc.sync.dma_start(va[:, 0:H], kv_sb[64:128, j * KC:(j + 1) * KC],
                                      transpose=True)
                    vaug.append(va)

            # ---- Q projection + attention per q-superblock ----
            for sg in range(n_sup):
                xq_sb = xqp.tile([128, CC, SUP], BF16)
                nc.sync.dma_start(xq_sb[:], xq_r[:, :, sg * SUP:(sg + 1) * SUP])
                psq = projp.tile([H, SUP], F32, tag="proj")
                for c in range(CC):
                    nc.tensor.matmul(psq[:], wq_sb[:, c, :], xq_sb[:, c, :],
                                     start=(c == 0), stop=(c == CC - 1))
                qT = pers.tile([H, SUP], BF16, tag=f"qT{sg}")
                nc.scalar.activation(qT[:], psq[:],
                                     mybir.ActivationFunctionType.Identity,
                                     bias=bq_sb[:], scale=1.0)

                n_loc = 2 * (sg + 1)
                ot_ps = otp.tile([H + 1, SUP], F32)
                for k in range(n_loc):
                    skv, off = k // 4, (k % 4) * KC
                    s_ps = sp.tile([128, SUP], F32)
                    nc.tensor.matmul(s_ps[:], kvT[skv][0:64, off:off + KC], qT[:],
                                     start=True, stop=True)
                    pt = ptp.tile([128, SUP], BF16)
                    nc.scalar.activation(pt[:], s_ps[:],
                                         mybir.ActivationFunctionType.Exp,
                                         scale=scale)
                    if k >= n_loc - 2:
                        m = k - (n_loc - 2)
                        nc.vector.tensor_mul(pt[:], pt[:], mask_sb[:, m, :])
                    nc.tensor.matmul(ot_ps[:], vaug[k][:], pt[:],
                                     start=(k == 0), stop=(k == n_loc - 1))

                ot_s = otsb.tile([H + 1, SUP], F32)
                nc.vector.tensor_copy(ot_s[:], ot_ps[:])
                stg = stgp.tile([128, SUP // KC, H + 1], F32)
                for qb in range(SUP // KC):
                    tp = tpp.tile([128, H + 1], F32)
                    nc.tensor.transpose(tp[:], ot_s[:, qb * KC:(qb + 1) * KC],
                                        id_sb[0:H + 1, 0:H + 1])
                    nc.vector.tensor_copy(stg[:, qb, :], tp[:])
                nc.sync.dma_start(
                    out_d[sg * SUP:(sg + 1) * SUP, :]
                    .rearrange("(qb p) n -> p qb n", p=128),
                    stg[:])
    nc.compile()
    return nc


def make_core_inputs(xT_b, wq_b, wkv_b, bq_c, bkv_c, ident, h, T):
    """Per-core input dict. xT_b: [C, T] bf16 for this core's batch."""
    gs = 2 * np.arange(T // (2 * KC)) + h
    cols = (gs[:, None] * KC + np.arange(KC)[None, :]).reshape(-1)
    xkv = np.ascontiguousarray(xT_b[:, cols])
    mask = np.zeros((128, 2, SUP), dtype=ml_dtypes.bfloat16)
    colv = np.arange(SUP)[None, :]
    for m in range(2):
        j = 2 * m + h
        mask[:, m, :] = ((j * KC + np.arange(128))[:, None] <= colv)
    return {"xq": xT_b, "xkv": xkv, "wq": wq_b, "wkv": wkv_b,
            "bq": bq_c, "bkv": bkv_c, "mask": mask, "ident": ident}


def prep_inputs(x, Wq, bq, Wk, bk, Wv, bv, T):
    xT = np.ascontiguousarray(
        np.transpose(np.asarray(x, np.float32), (0, 2, 1))).astype(ml_dtypes.bfloat16)
    wq_b = np.asarray(Wq, np.float32).astype(ml_dtypes.bfloat16)
    wkv_b = np.concatenate([np.asarray(Wk, np.float32),
                            np.asarray(Wv, np.float32)], 1).astype(ml_dtypes.bfloat16)
    bq_c = np.asarray(bq, np.float32).reshape(H, 1).copy()
    bkv_c = np.concatenate([np.asarray(bk, np.float32),
                            np.asarray(bv, np.float32)]).reshape(2 * H, 1).copy()
    ident = np.eye(128, dtype=np.float32)
    n_b = xT.shape[0]
    return [make_core_inputs(xT[c // 2], wq_b, wkv_b, bq_c, bkv_c, ident,
                             c % 2, T)
            for c in range(2 * n_b)]


def combine(results, T):
    n_b = len(results) // 2
    out = np.empty((n_b, T, H), np.float32)
    for b in range(n_b):
        r0 = results[2 * b]["out"].astype(np.float64)
        r1 = results[2 * b + 1]["out"].astype(np.float64)
        num = r0[:, :H] + r1[:, :H]
        den = r0[:, H:] + r1[:, H:]
        out[b] = (num / den).astype(np.float32)
    return out


_NC = None


def kernel(x, Wq, bq, Wk, bk, Wv, bv):
    global _NC
    T = np.asarray(x).shape[1]
    if _NC is None:
        _NC = build_nc(T)
    in_maps = prep_inputs(x, Wq, bq, Wk, bk, Wv, bv, T)
    res = run_bass_kernel_spmd(_NC, in_maps, core_ids=list(range(8)))
    return combine(res.results, T)
